# revision 9
# baseline (speedup 1.0000x reference)
"""ConformerBlock Trainium2 kernel (v2).

Sharding: data-parallel over batch. B=16 -> 2 batch elements per core x 8 cores.
Weights replicated, no collectives.

v2 changes vs baseline:
- LayerNorm stats broadcast-first: mean/E[x^2] computed as (J/D) @ x matmuls
  straight into [128, NT] PSUM (no slow single-partition row ops).
- All big weights bf16 (half SBUF/DMA); double-buffered weight pools so the
  next stage's weights prefetch during the current stage.
- Rel-pos S matrix computed over the needed 640-wide window per n-tile
  (instead of full 1024), still staged via DRAM skewed-AP gather.
- Depthwise conv split: 21 taps as diagonal matmuls on PE (diagonals built on
  the idle GpSimd engine), 10 even-offset taps as fused multiply-add chains on
  the vector engine (bf16 2x mode).
- dots / S / attnv matmuls use 64-contraction row/col tiles so head pairs run
  concurrently in the PE array.
- Explicit engine routing (vector for copies, scalar for activations) to avoid
  nc.any landing copies on the scalar engine.
"""
import numpy as np
import ml_dtypes

import concourse.bass as bass
import concourse.bacc as bacc
import concourse.tile as tile
from concourse import mybir
from concourse.bass_utils import run_bass_kernel_spmd
from concourse.masks import make_identity

F32 = mybir.dt.float32
F32R = mybir.dt.float32r
BF16 = mybir.dt.bfloat16
AF = mybir.ActivationFunctionType
OP = mybir.AluOpType
BFNP = ml_dtypes.bfloat16

DEBUG_TAPS = False
B, NT, D = 16, 512, 512
H, DH = 8, 64
INNER = H * DH                    # 512
FF = 4 * D                        # 2048
CI = 2 * D                        # 1024
KW = 31
EPS = 1e-5
P = 128
NCORES = 8
BPC = B // NCORES                 # 2
KD = D // P                       # 4
KF = FF // P                      # 16
KC = CI // P                      # 8
SW = 640                          # rel-pos window width per n-tile
HGW = NT + 30                     # padded GLU tile width (15 + 512 + 15)

# depthwise-conv tap split: DVE takes even offsets (4B-aligned bf16 starts)
DVE_TAPS = list(range(0, 20, 2))            # 10 taps on vector engine
PE_TAPS = [k for k in range(KW) if k not in DVE_TAPS]   # 21 taps on PE


# --------------------------------------------------------------------------
# host-side weight folding
# --------------------------------------------------------------------------

def _host_prepare(inp):
    g = {k: np.ascontiguousarray(np.asarray(v, np.float32)) for k, v in inp.items()}
    p = {}
    scale = DH ** (-0.5)

    p['w_ff1_1'] = g['ff1_w1'] * g['ff1_ln_g'][:, None]
    p['b_ff1_1'] = g['ff1_b1'] + g['ff1_ln_b'] @ g['ff1_w1']
    p['w_ff1_2'] = 0.5 * g['ff1_w2']
    p['b_ff1_2'] = 0.5 * g['ff1_b2']
    p['w_ff2_1'] = g['ff2_w1'] * g['ff2_ln_g'][:, None]
    p['b_ff2_1'] = g['ff2_b1'] + g['ff2_ln_b'] @ g['ff2_w1']
    p['w_ff2_2'] = 0.5 * g['ff2_w2']
    p['b_ff2_2'] = 0.5 * g['ff2_b2']

    qkv_w = g['qkv_w'] * g['attn_ln_g'][:, None]
    qkv_b = g['attn_ln_b'] @ g['qkv_w']
    qkv_w[:, :INNER] *= scale
    qkv_b[:INNER] *= scale
    p['qkv_w'] = qkv_w
    p['qkv_b'] = qkv_b
    p['qkv_bv'] = np.ascontiguousarray(qkv_b[2 * INNER:][None, :])
    p['out_w'] = g['out_w']
    rm = g['rel_emb'][:1024][::-1].T                       # [DH, 1024]
    p['r_mat'] = np.ascontiguousarray(np.concatenate([rm, rm], 0))  # [128, 1024]

    pw1 = g['pw1_w'] * g['conv_ln_g'][None, :]
    p['pw1_wT'] = np.ascontiguousarray(pw1.T)
    p['b_pw1'] = g['pw1_b'] + pw1 @ g['conv_ln_b']
    bnsc = g['bn_g'] / np.sqrt(g['bn_var'] + EPS)
    p['dw_w'] = g['dw_w'][:, 0, :] * bnsc[:, None]
    p['dw_b'] = (g['dw_b'] - g['bn_mean']) * bnsc + g['bn_b']
    p['pw2_wT'] = np.ascontiguousarray(g['pw2_w'].T)
    p['b_pw2'] = g['pw2_b']
    p['post_g'] = g['post_ln_g']
    p['post_b'] = g['post_ln_b']
    return p


def _col(v, nch):
    return np.ascontiguousarray(v.reshape(nch, P).T)


# --------------------------------------------------------------------------
# device program
# --------------------------------------------------------------------------

def _build_nc():
    nc = bacc.Bacc("TRN2", target_bir_lowering=False, debug=False, num_devices=1)

    def par(name, shape, dt=BF16, out=False):
        return nc.dram_tensor(name, list(shape), dt,
                              kind="ExternalOutput" if out else "ExternalInput").ap()

    pr = {}
    pr["xT"] = par("xT", [BPC, D, NT], F32R)
    pr["yO"] = par("y", [BPC, D, NT], F32, out=True)
    pr["w_ff1_1"] = par("w_ff1_1", [D, FF])
    pr["w_ff1_2"] = par("w_ff1_2", [FF, D])
    pr["w_ff2_1"] = par("w_ff2_1", [D, FF])
    pr["w_ff2_2"] = par("w_ff2_2", [FF, D])
    pr["qkv_w"] = par("qkv_w", [D, 3 * INNER])
    pr["out_w"] = par("out_w", [INNER, D])
    pr["r_mat"] = par("r_mat", [P, 1024])
    pr["pw1_wT"] = par("pw1_wT", [D, 2 * CI])
    pr["pw2_wT"] = par("pw2_wT", [CI, D])
    pr["qkv_bv"] = par("qkv_bv", [1, INNER], F32R)
    pr["b_ff1_1"] = par("b_ff1_1", [P, KF], F32)
    pr["b_ff1_2"] = par("b_ff1_2", [P, KD], F32)
    pr["b_ff2_1"] = par("b_ff2_1", [P, KF], F32)
    pr["b_ff2_2"] = par("b_ff2_2", [P, KD], F32)
    pr["qkv_b"] = par("qkv_b", [P, 8], F32)
    pr["b_pw1"] = par("b_pw1", [P, 2 * KC], F32)
    pr["dw_w"] = par("dw_w", [P, KC, KW], F32)
    pr["dw_b"] = par("dw_b", [P, KC], F32)
    pr["b_pw2"] = par("b_pw2", [P, KD], F32)
    pr["post_g"] = par("post_g", [P, KD], F32)
    pr["post_b"] = par("post_b", [P, KD], F32)
    if DEBUG_TAPS:
        for i in range(1, 5):
            pr[f"dbg{i}"] = par(f"dbg{i}", [BPC, D, NT], F32, out=True)

    with tile.TileContext(nc) as tc:
        _emit(nc, tc, pr)
    nc.compile()
    return nc


def _emit(nc, tc, pr):
    from contextlib import ExitStack
    ctx = ExitStack()
    with ctx:
        sing = ctx.enter_context(tc.tile_pool(name="sing", bufs=1))
        sb = ctx.enter_context(tc.tile_pool(name="sb", bufs=2))
        ps_ = ctx.enter_context(tc.tile_pool(name="ps", bufs=1, space="PSUM"))
        dram = ctx.enter_context(tc.tile_pool(name="dram", bufs=4, space="DRAM"))

        def st(shape, dt, tag, bufs, name):
            return sb.tile(list(shape), dt, tag=tag, bufs=bufs, name=name)

        def pt(shape, dt, tag, bufs, name):
            return ps_.tile(list(shape), dt, tag=tag, bufs=bufs, name=name)

        def mm_ps(name, w=NT):
            return pt([P, w], F32, "mm", 4, name)

        # ---- constants ----
        ident_bf = sing.tile([P, P], BF16)
        make_identity(nc, ident_bf)
        onesJ = sing.tile([P, P], BF16)
        nc.vector.memset(onesJ, 1.0 / D)
        onesJf = sing.tile([P, P], F32)
        nc.vector.memset(onesJf, 1.0 / D)
        onesJr = onesJf.bitcast(F32R)
        ones1_f = sing.tile([1, P], F32)
        nc.vector.memset(ones1_f, 1.0)
        ones1_r = sing.tile([1, P], F32R)
        nc.vector.tensor_copy(ones1_r, ones1_f)
        eps_t = sing.tile([P, 1], F32)
        nc.vector.memset(eps_t, EPS)
        zero16 = sing.tile([P, 16], BF16)
        nc.vector.memset(zero16, 0.0)

        def load_small(name, shape, dt=F32):
            t = sing.tile(list(shape), dt, name=f"sb_{name}")
            nc.sync.dma_start(t[:], pr[name][:])
            return t

        sb_bff11 = load_small("b_ff1_1", [P, KF])
        sb_bff12 = load_small("b_ff1_2", [P, KD])
        sb_bff21 = load_small("b_ff2_1", [P, KF])
        sb_bff22 = load_small("b_ff2_2", [P, KD])
        sb_qkvb = load_small("qkv_b", [P, 8])
        sb_qkvbv = load_small("qkv_bv", [1, INNER], F32R)
        sb_bpw1 = load_small("b_pw1", [P, 2 * KC])
        sb_dww = load_small("dw_w", [P, KC, KW])
        sb_dwb = load_small("dw_b", [P, KC])
        sb_bpw2 = load_small("b_pw2", [P, KD])
        sb_postg = load_small("post_g", [P, KD])
        sb_postb = load_small("post_b", [P, KD])
        sb_rmat = load_small("r_mat", [P, 1024], BF16)

        def load_w(ap, ktiles, fdim, tag, name):
            """Load [ktiles*P, fdim] weights as two half-k tiles (smaller pool
            slots so double-buffered prefetch fits SBUF). Returns an indexable
            shim: w[:, k, a:b]."""
            kh = ktiles // 2
            src = ap.rearrange("(k p) f -> p k f", p=P)
            halves = []
            for hlf in range(2):
                t = st([P, kh, fdim], BF16, tag, 3, f"{name}_{hlf}")
                for k in range(kh):
                    nc.scalar.dma_start(t[:, k, :], src[:, hlf * kh + k, :])
                halves.append(t)

            class _W:
                def __getitem__(self, idx):
                    _, k, fs = idx
                    return halves[k // kh][:, k % kh, fs]
            return _W()

        # ---- input ----
        x = {}
        for b in range(BPC):
            tiles = []
            for k in range(KD):
                t = st([P, NT], F32R, "xcur", 10, f"x0_{b}_{k}")
                nc.sync.dma_start(t[:], pr["xT"][b, k * P:(k + 1) * P, :])
                tiles.append(t)
            x[b] = tiles

        # ---- layernorm: broadcast-first stats ----
        def ln_full(b, pfx):
            sq = []
            for k in range(KD):
                s = st([P, NT], BF16, "sq", 4, f"sq{pfx}{b}_{k}")
                nc.scalar.activation(s, x[b][k].bitcast(F32), AF.Square)
                sq.append(s)
            mean_ps = mm_ps(f"mean{pfx}{b}")
            for k in range(KD):
                nc.tensor.matmul(mean_ps, onesJr, x[b][k],
                                 start=(k == 0), stop=(k == KD - 1))
            ex2_ps = mm_ps(f"ex2{pfx}{b}")
            for k in range(KD):
                nc.tensor.matmul(ex2_ps, onesJ, sq[k],
                                 start=(k == 0), stop=(k == KD - 1))
            mb = st([P, NT], BF16, "mb", 2, f"mb{pfx}{b}")
            nc.vector.tensor_copy(mb, mean_ps)
            m2 = st([P, NT], BF16, "m2", 2, f"m2{pfx}{b}")
            nc.vector.tensor_mul(m2, mb, mb)
            var = st([P, NT], F32, "var", 2, f"var{pfx}{b}")
            nc.vector.tensor_tensor(var, ex2_ps, m2, OP.subtract)
            nc.scalar.activation(var, var, AF.Sqrt, bias=eps_t[:, 0:1])
            rinv = st([P, NT], F32, "rinv", 2, f"rinv{pfx}{b}")
            nc.vector.reciprocal(rinv, var)
            return mb, rinv

        def ln_apply(b, mb, rinv, pfx):
            hs = []
            for k in range(KD):
                t = st([P, NT], BF16, "t2k", 3, f"t{pfx}{b}_{k}")
                nc.vector.tensor_tensor(t, x[b][k].bitcast(F32), mb, OP.subtract)
                h = st([P, NT], BF16, "h", 8, f"h{pfx}{b}_{k}")
                nc.vector.tensor_mul(h, t, rinv)
                hs.append(h)
            return hs

        # ---- feed-forward ----
        def ff_stage(w1name, w2name, b1t, b2t, pfx):
            w1 = load_w(pr[w1name], KD, FF, "wbig", f"w1{pfx}")
            w2 = load_w(pr[w2name], KF, D, "wmid", f"w2{pfx}")
            hh = {}
            for b in range(BPC):
                mb, rinv = ln_full(b, pfx)
                hh[b] = ln_apply(b, mb, rinv, pfx)
            for b in range(BPC):
                hs = hh[b]
                y1 = st([P, KF, NT], BF16, "y1s", 1, f"y1s{pfx}{b}")
                for f in range(KF):
                    ps = mm_ps(f"ps1{pfx}{b}_{f}")
                    for k in range(KD):
                        nc.tensor.matmul(ps, w1[:, k, f * P:(f + 1) * P], hs[k],
                                         start=(k == 0), stop=(k == KD - 1))
                    nc.scalar.activation(y1[:, f, :], ps, AF.Silu,
                                         bias=b1t[:, f:f + 1])
                newx = []
                for f in range(KD):
                    ps = mm_ps(f"ps2{pfx}{b}_{f}")
                    for k in range(KF):
                        nc.tensor.matmul(ps, w2[:, k, f * P:(f + 1) * P],
                                         y1[:, k, :],
                                         start=(k == 0), stop=(k == KF - 1))
                    nx = st([P, NT], F32R, "xcur", 10, f"x{pfx}{b}_{f}")
                    nc.vector.scalar_tensor_tensor(
                        nx, ps, b2t[:, f:f + 1], x[b][f].bitcast(F32),
                        OP.add, OP.add)
                    newx.append(nx)
                x[b] = newx

        # ---- attention ----
        def attn_stage():
            wq = load_w(pr["qkv_w"], KD, 3 * INNER, "wbig", "wqkv")
            wo = load_w(pr["out_w"], KD, D, "wmid", "wout")
            hh = {}
            for b in range(BPC):
                mb, rinv = ln_full(b, "at")
                hh[b] = ln_apply(b, mb, rinv, "at")
            for b in range(BPC):
                hs = hh[b]
                qk = st([P, 8, NT], BF16, "qk", 1, f"qk{b}")
                for f in range(8):
                    ps = mm_ps(f"qkps{b}_{f}")
                    for k in range(KD):
                        nc.tensor.matmul(ps, wq[:, k, f * P:(f + 1) * P], hs[k],
                                         start=(k == 0), stop=(k == KD - 1))
                    nc.vector.tensor_scalar(qk[:, f, :], ps,
                                            sb_qkvb[:, f:f + 1], None, OP.add)
                vt = st([P, KD, INNER], BF16, "vt", 2, f"vt{b}")
                for n in range(KD):
                    ps = mm_ps(f"vps{b}_{n}")
                    for k in range(KD):
                        nc.tensor.matmul(ps, hs[k][:, n * P:(n + 1) * P],
                                         wq[:, k, 2 * INNER:3 * INNER],
                                         start=(k == 0), stop=False)
                    nc.tensor.matmul(ps, ones1_r[0:1, :], sb_qkvbv,
                                     start=False, stop=True)
                    nc.vector.tensor_copy(vt[:, n, :], ps)
                ao = st([P, KD, NT], BF16, "ao", 2, f"ao{b}")
                for hp in range(4):
                    qt = qk[:, hp, :]
                    kt = qk[:, 4 + hp, :]
                    # --- rel-pos S over 640-wide windows, staged via DRAM ---
                    Sd = {}
                    for eo in range(2):
                        Sd[eo] = dram.tile([NT, SW], BF16, tag="Sd",
                                           name=f"Sd{b}_{hp}_{eo}")
                    for mi in range(KD):
                        w0 = 384 - 128 * mi
                        for eo in range(2):
                            po = eo * DH
                            sm = mm_ps(f"sm{b}_{hp}_{mi}_{eo}")
                            nc.tensor.matmul(
                                sm, qt[po:po + DH, mi * P:(mi + 1) * P],
                                sb_rmat[po:po + DH, w0:w0 + 512],
                                start=True, stop=True)
                            se = pt([P, SW - 512], F32, "mm", 4,
                                    f"se{b}_{hp}_{mi}_{eo}")
                            nc.tensor.matmul(
                                se, qt[po:po + DH, mi * P:(mi + 1) * P],
                                sb_rmat[po:po + DH, w0 + 512:w0 + SW],
                                start=True, stop=True)
                            sbf = st([P, SW], BF16, "sbf", 3,
                                     f"sbf{b}_{hp}_{mi}_{eo}")
                            nc.scalar.activation(sbf[:, :512], sm, AF.Copy)
                            nc.scalar.activation(sbf[:, 512:], se, AF.Copy)
                            nc.sync.dma_start(
                                Sd[eo][mi * P:(mi + 1) * P, :], sbf[:])
                    # --- dots + softmax (heads run in 64-row PE tiles) ---
                    attn = {}
                    sums = {}
                    for eo in range(2):
                        attn[eo] = st([P, KD, NT], BF16, "attn", 2,
                                      f"at{b}_{hp}_{eo}")
                        sums[eo] = st([P, KD], F32, "sums", 6,
                                      f"sums{b}_{hp}_{eo}")
                    for mi in range(KD):
                        dps = {}
                        for eo in range(2):
                            po = eo * DH
                            dp = mm_ps(f"dots{b}_{hp}_{mi}_{eo}")
                            nc.tensor.matmul(
                                dp, qt[po:po + DH, mi * P:(mi + 1) * P],
                                kt[po:po + DH, :], start=True, stop=True)
                            dps[eo] = dp
                        for eo in range(2):
                            pos = st([P, NT], BF16, "pos", 3,
                                     f"pos{b}_{hp}_{mi}_{eo}")
                            skew = bass.AP(
                                tensor=Sd[eo].tensor,
                                offset=Sd[eo].offset + mi * P * SW + 127,
                                ap=[[SW - 1, P], [1, NT]])
                            nc.sync.dma_start(pos[:], skew)
                            sc = st([P, NT], BF16, "sc", 3,
                                    f"sc{b}_{hp}_{mi}_{eo}")
                            nc.vector.tensor_tensor(sc, dps[eo], pos, OP.add)
                            nc.scalar.activation(
                                attn[eo][:, mi, :], sc, AF.Exp,
                                accum_out=sums[eo][:, mi:mi + 1])
                    opsb = pt([P, NT], F32, "av", 2, f"ops{b}_{hp}")
                    for eo in range(2):
                        po = eo * DH
                        h = 2 * hp + eo
                        rec = st([P, KD], F32, "sums", 6, f"rec{b}_{hp}_{eo}")
                        nc.vector.reciprocal(rec, sums[eo])
                        for mi in range(KD):
                            nc.vector.tensor_scalar_mul(
                                attn[eo][:, mi, :], attn[eo][:, mi, :],
                                rec[:, mi:mi + 1])
                        attT = st([P, KD, NT], BF16, "attT", 2,
                                  f"attT{b}_{hp}_{eo}")
                        for ki in range(KD):
                            tps = pt([P, NT], BF16, "tr", 2,
                                     f"tr{b}_{hp}_{eo}_{ki}")
                            for mi in range(KD):
                                nc.tensor.transpose(
                                    tps[:, mi * P:(mi + 1) * P],
                                    attn[eo][:, mi, ki * P:(ki + 1) * P],
                                    ident_bf)
                            nc.vector.tensor_copy(attT[:, ki, :], tps)
                            nc.tensor.matmul(
                                opsb[po:po + DH, :],
                                vt[:, ki, h * DH:(h + 1) * DH],
                                attT[:, ki, :],
                                start=(ki == 0), stop=(ki == KD - 1))
                    nc.vector.tensor_copy(ao[:, hp, :], opsb)
                newx = []
                for f in range(KD):
                    ps = mm_ps(f"oproj{b}_{f}")
                    for k in range(KD):
                        nc.tensor.matmul(ps, wo[:, k, f * P:(f + 1) * P],
                                         ao[:, k, :],
                                         start=(k == 0), stop=(k == KD - 1))
                    nx = st([P, NT], F32R, "xcur", 10, f"xat{b}_{f}")
                    nc.vector.tensor_tensor(nx, ps, x[b][f].bitcast(F32), OP.add)
                    newx.append(nx)
                x[b] = newx

        # ---- conv module ----
        def conv_stage():
            w1 = load_w(pr["pw1_wT"], KD, 2 * CI, "wbig", "wpw1")
            w2 = load_w(pr["pw2_wT"], KC, D, "wmid", "wpw2")
            hh = {}
            for b in range(BPC):
                mb, rinv = ln_full(b, "cv")
                hh[b] = ln_apply(b, mb, rinv, "cv")
            hc = {b: st([P, KC, NT], BF16, "hc", 2, f"hc{b}")
                  for b in range(BPC)}
            for c in range(KC):
                hg = {}
                for b in range(BPC):
                    hs = hh[b]
                    hgb = st([P, HGW], BF16, "hglu", 6, f"hglu{b}_{c}")
                    nc.vector.tensor_copy(hgb[:, 0:15], zero16[:, 0:15])
                    nc.vector.tensor_copy(hgb[:, NT + 15:], zero16[:, 0:HGW - NT - 15])
                    pso = mm_ps(f"glo{b}_{c}")
                    for k in range(KD):
                        nc.tensor.matmul(pso, w1[:, k, c * P:(c + 1) * P], hs[k],
                                         start=(k == 0), stop=(k == KD - 1))
                    psg = mm_ps(f"glg{b}_{c}")
                    for k in range(KD):
                        nc.tensor.matmul(psg,
                                         w1[:, k, CI + c * P:CI + (c + 1) * P],
                                         hs[k],
                                         start=(k == 0), stop=(k == KD - 1))
                    sg = st([P, NT], BF16, "sg", 3, f"sig{b}_{c}")
                    nc.scalar.activation(sg, psg, AF.Sigmoid,
                                         bias=sb_bpw1[:, KC + c:KC + c + 1])
                    nc.vector.scalar_tensor_tensor(
                        hgb[:, 15:NT + 15], pso, sb_bpw1[:, c:c + 1], sg,
                        OP.add, OP.mult)
                    hg[b] = hgb
                diags = {}
                for k in PE_TAPS:
                    d = st([P, P], BF16, "diag", 24, f"dg{c}_{k}")
                    nc.gpsimd.tensor_scalar_mul(d, ident_bf,
                                                sb_dww[:, c, k:k + 1])
                    diags[k] = d
                for b in range(BPC):
                    hgb = hg[b]
                    ps = mm_ps(f"cv{b}_{c}")
                    for i, k in enumerate(PE_TAPS):
                        nc.tensor.matmul(ps, diags[k], hgb[:, k:k + NT],
                                         start=(i == 0),
                                         stop=(i == len(PE_TAPS) - 1))
                    acc = st([P, NT], BF16, "acc", 3, f"acc{b}_{c}")
                    k0 = DVE_TAPS[0]
                    nc.vector.tensor_scalar_mul(acc, hgb[:, k0:k0 + NT],
                                                sb_dww[:, c, k0:k0 + 1])
                    for k in DVE_TAPS[1:]:
                        nc.vector.scalar_tensor_tensor(
                            acc, hgb[:, k:k + NT], sb_dww[:, c, k:k + 1], acc,
                            OP.mult, OP.add)
                    tt = st([P, NT], BF16, "sg", 3, f"cvt{b}_{c}")
                    nc.vector.scalar_tensor_tensor(
                        tt, ps, sb_dwb[:, c:c + 1], acc, OP.add, OP.add)
                    nc.scalar.activation(hc[b][:, c, :], tt, AF.Silu)
            for b in range(BPC):
                newx = []
                for f in range(KD):
                    ps = mm_ps(f"pw2{b}_{f}")
                    for k in range(KC):
                        nc.tensor.matmul(ps, w2[:, k, f * P:(f + 1) * P],
                                         hc[b][:, k, :],
                                         start=(k == 0), stop=(k == KC - 1))
                    nx = st([P, NT], F32R, "xcur", 10, f"xcv{b}_{f}")
                    nc.vector.scalar_tensor_tensor(
                        nx, ps, sb_bpw2[:, f:f + 1], x[b][f].bitcast(F32),
                        OP.add, OP.add)
                    newx.append(nx)
                x[b] = newx

        # ---- post layernorm ----
        def post_stage():
            for b in range(BPC):
                mb, rinv = ln_full(b, "po")
                for f in range(KD):
                    t = st([P, NT], BF16, "t2k", 3, f"pt{b}_{f}")
                    nc.vector.tensor_tensor(t, x[b][f].bitcast(F32), mb,
                                            OP.subtract)
                    t2 = st([P, NT], BF16, "sg", 3, f"pt2{b}_{f}")
                    nc.vector.tensor_mul(t2, t, rinv)
                    yt = st([P, NT], F32, "yout", 2, f"y{b}_{f}")
                    nc.vector.tensor_scalar(yt, t2, sb_postg[:, f:f + 1],
                                            sb_postb[:, f:f + 1],
                                            OP.mult, OP.add)
                    nc.sync.dma_start(pr["yO"][b, f * P:(f + 1) * P, :], yt[:])

        def tap(i):
            if not DEBUG_TAPS:
                return
            for b in range(BPC):
                for f in range(KD):
                    nc.sync.dma_start(pr[f"dbg{i}"][b, f * P:(f + 1) * P, :],
                                      x[b][f].bitcast(F32)[:])

        ff_stage("w_ff1_1", "w_ff1_2", sb_bff11, sb_bff12, "f1")
        tap(1)
        attn_stage()
        tap(2)
        conv_stage()
        tap(3)
        ff_stage("w_ff2_1", "w_ff2_2", sb_bff21, sb_bff22, "f2")
        tap(4)
        post_stage()


# --------------------------------------------------------------------------
# host entry point
# --------------------------------------------------------------------------

_NC = None


def _get_nc():
    global _NC
    if _NC is None:
        _NC = _build_nc()
    return _NC


def _shared_maps(p):
    return {
        'w_ff1_1': p['w_ff1_1'].astype(BFNP),
        'w_ff1_2': p['w_ff1_2'].astype(BFNP),
        'w_ff2_1': p['w_ff2_1'].astype(BFNP),
        'w_ff2_2': p['w_ff2_2'].astype(BFNP),
        'qkv_w': p['qkv_w'].astype(BFNP),
        'out_w': p['out_w'].astype(BFNP),
        'r_mat': p['r_mat'].astype(BFNP),
        'pw1_wT': p['pw1_wT'].astype(BFNP),
        'pw2_wT': p['pw2_wT'].astype(BFNP),
        'qkv_bv': p['qkv_bv'],
        'b_ff1_1': _col(p['b_ff1_1'], KF), 'b_ff1_2': _col(p['b_ff1_2'], KD),
        'b_ff2_1': _col(p['b_ff2_1'], KF), 'b_ff2_2': _col(p['b_ff2_2'], KD),
        'qkv_b': _col(p['qkv_b'][:2 * INNER], 8),
        'b_pw1': _col(p['b_pw1'], 2 * KC),
        'dw_w': np.ascontiguousarray(
            p['dw_w'].reshape(KC, P, KW).transpose(1, 0, 2)),
        'dw_b': _col(p['dw_b'], KC),
        'b_pw2': _col(p['b_pw2'], KD),
        'post_g': _col(p['post_g'], KD), 'post_b': _col(p['post_b'], KD),
    }


def kernel(**inputs):
    p = _host_prepare(inputs)
    x = np.asarray(inputs['x'], np.float32)
    shared = _shared_maps(p)
    in_maps = []
    for c in range(NCORES):
        m = dict(shared)
        xb = x[c * BPC:(c + 1) * BPC]
        m['xT'] = np.ascontiguousarray(xb.transpose(0, 2, 1))
        in_maps.append(m)

    nc = _get_nc()
    res = run_bass_kernel_spmd(nc, in_maps, core_ids=list(range(NCORES)))
    out = np.empty((B, NT, D), np.float32)
    for c in range(NCORES):
        yT = res.results[c]['y']
        out[c * BPC:(c + 1) * BPC] = yT.transpose(0, 2, 1)
    return out


# revision 12
# speedup vs baseline: 1.3617x; 1.3617x over previous
"""ConformerBlock Trainium2 kernel (v2).

Sharding: data-parallel over batch. B=16 -> 2 batch elements per core x 8 cores.
Weights replicated, no collectives.

v2 changes vs baseline:
- LayerNorm stats broadcast-first: mean/E[x^2] computed as (J/D) @ x matmuls
  straight into [128, NT] PSUM (no slow single-partition row ops).
- All big weights bf16 (half SBUF/DMA); double-buffered weight pools so the
  next stage's weights prefetch during the current stage.
- Rel-pos S matrix computed over the needed 640-wide window per n-tile
  (instead of full 1024), still staged via DRAM skewed-AP gather.
- Depthwise conv split: 21 taps as diagonal matmuls on PE (diagonals built on
  the idle GpSimd engine), 10 even-offset taps as fused multiply-add chains on
  the vector engine (bf16 2x mode).
- dots / S / attnv matmuls use 64-contraction row/col tiles so head pairs run
  concurrently in the PE array.
- Explicit engine routing (vector for copies, scalar for activations) to avoid
  nc.any landing copies on the scalar engine.
"""
import numpy as np
import ml_dtypes

import concourse.bass as bass
import concourse.bacc as bacc
import concourse.tile as tile
from concourse import mybir
from concourse.bass_utils import run_bass_kernel_spmd
from concourse.masks import make_identity

F32 = mybir.dt.float32
F32R = mybir.dt.float32r
BF16 = mybir.dt.bfloat16
AF = mybir.ActivationFunctionType
OP = mybir.AluOpType
BFNP = ml_dtypes.bfloat16

DEBUG_TAPS = False
B, NT, D = 16, 512, 512
H, DH = 8, 64
INNER = H * DH                    # 512
FF = 4 * D                        # 2048
CI = 2 * D                        # 1024
KW = 31
EPS = 1e-5
P = 128
NCORES = 8
BPC = B // NCORES                 # 2
KD = D // P                       # 4
KF = FF // P                      # 16
KC = CI // P                      # 8
SW = 640                          # rel-pos window width per n-tile
HGW = NT + 30                     # padded GLU tile width (15 + 512 + 15)

# depthwise-conv taps all run as diagonal matmuls on PE; the diagonal
# weight matrices are built on scalar/vector engines (alternating).
PE_TAPS = list(range(KW))


# --------------------------------------------------------------------------
# host-side weight folding
# --------------------------------------------------------------------------

def _host_prepare(inp):
    g = {k: np.ascontiguousarray(np.asarray(v, np.float32)) for k, v in inp.items()}
    p = {}
    scale = DH ** (-0.5)

    p['w_ff1_1'] = g['ff1_w1'] * g['ff1_ln_g'][:, None]
    p['b_ff1_1'] = g['ff1_b1'] + g['ff1_ln_b'] @ g['ff1_w1']
    p['w_ff1_2'] = 0.5 * g['ff1_w2']
    p['b_ff1_2'] = 0.5 * g['ff1_b2']
    p['w_ff2_1'] = g['ff2_w1'] * g['ff2_ln_g'][:, None]
    p['b_ff2_1'] = g['ff2_b1'] + g['ff2_ln_b'] @ g['ff2_w1']
    p['w_ff2_2'] = 0.5 * g['ff2_w2']
    p['b_ff2_2'] = 0.5 * g['ff2_b2']

    qkv_w = g['qkv_w'] * g['attn_ln_g'][:, None]
    qkv_b = g['attn_ln_b'] @ g['qkv_w']
    qkv_w[:, :INNER] *= scale
    qkv_b[:INNER] *= scale
    p['qkv_w'] = qkv_w
    p['qkv_b'] = qkv_b
    p['qkv_bv'] = np.ascontiguousarray(qkv_b[2 * INNER:][None, :])
    p['out_w'] = g['out_w']
    rm = g['rel_emb'][:1024][::-1].T                       # [DH, 1024]
    p['r_mat'] = np.ascontiguousarray(np.concatenate([rm, rm], 0))  # [128, 1024]

    pw1 = g['pw1_w'] * g['conv_ln_g'][None, :]
    p['pw1_wT'] = np.ascontiguousarray(pw1.T)
    p['b_pw1'] = g['pw1_b'] + pw1 @ g['conv_ln_b']
    bnsc = g['bn_g'] / np.sqrt(g['bn_var'] + EPS)
    p['dw_w'] = g['dw_w'][:, 0, :] * bnsc[:, None]
    p['dw_b'] = (g['dw_b'] - g['bn_mean']) * bnsc + g['bn_b']
    p['pw2_wT'] = np.ascontiguousarray(g['pw2_w'].T)
    p['b_pw2'] = g['pw2_b']
    p['post_g'] = g['post_ln_g']
    p['post_b'] = g['post_ln_b']
    return p


def _col(v, nch):
    return np.ascontiguousarray(v.reshape(nch, P).T)


# --------------------------------------------------------------------------
# device program
# --------------------------------------------------------------------------

def _build_nc():
    nc = bacc.Bacc("TRN2", target_bir_lowering=False, debug=False, num_devices=1)

    def par(name, shape, dt=BF16, out=False):
        return nc.dram_tensor(name, list(shape), dt,
                              kind="ExternalOutput" if out else "ExternalInput").ap()

    pr = {}
    pr["xT"] = par("xT", [BPC, D, NT], F32R)
    pr["yO"] = par("y", [BPC, D, NT], F32, out=True)
    pr["w_ff1_1"] = par("w_ff1_1", [D, FF])
    pr["w_ff1_2"] = par("w_ff1_2", [FF, D])
    pr["w_ff2_1"] = par("w_ff2_1", [D, FF])
    pr["w_ff2_2"] = par("w_ff2_2", [FF, D])
    pr["qkv_w"] = par("qkv_w", [D, 3 * INNER])
    pr["out_w"] = par("out_w", [INNER, D])
    pr["r_mat"] = par("r_mat", [P, 1024])
    pr["pw1_wT"] = par("pw1_wT", [D, 2 * CI])
    pr["pw2_wT"] = par("pw2_wT", [CI, D])
    pr["qkv_bv"] = par("qkv_bv", [1, INNER], F32R)
    pr["b_ff1_1"] = par("b_ff1_1", [P, KF], F32)
    pr["b_ff1_2"] = par("b_ff1_2", [P, KD], F32)
    pr["b_ff2_1"] = par("b_ff2_1", [P, KF], F32)
    pr["b_ff2_2"] = par("b_ff2_2", [P, KD], F32)
    pr["qkv_b"] = par("qkv_b", [P, 8], F32)
    pr["b_pw1"] = par("b_pw1", [P, 2 * KC], F32)
    pr["dw_w"] = par("dw_w", [P, KC, KW], F32)
    pr["dw_b"] = par("dw_b", [P, KC], F32)
    pr["b_pw2"] = par("b_pw2", [P, KD], F32)
    pr["post_g"] = par("post_g", [P, KD], F32)
    pr["post_b"] = par("post_b", [P, KD], F32)
    if DEBUG_TAPS:
        for i in range(1, 5):
            pr[f"dbg{i}"] = par(f"dbg{i}", [BPC, D, NT], F32, out=True)

    with tile.TileContext(nc) as tc:
        _emit(nc, tc, pr)
    nc.compile()
    return nc


def _emit(nc, tc, pr):
    from contextlib import ExitStack
    ctx = ExitStack()
    with ctx:
        sing = ctx.enter_context(tc.tile_pool(name="sing", bufs=1))
        sb = ctx.enter_context(tc.tile_pool(name="sb", bufs=2))
        ps_ = ctx.enter_context(tc.tile_pool(name="ps", bufs=1, space="PSUM"))
        dram = ctx.enter_context(tc.tile_pool(name="dram", bufs=4, space="DRAM"))

        def st(shape, dt, tag, bufs, name):
            return sb.tile(list(shape), dt, tag=tag, bufs=bufs, name=name)

        def pt(shape, dt, tag, bufs, name):
            return ps_.tile(list(shape), dt, tag=tag, bufs=bufs, name=name)

        def mm_ps(name, w=NT):
            return pt([P, w], F32, "mm", 4, name)

        # ---- constants ----
        ident_bf = sing.tile([P, P], BF16)
        make_identity(nc, ident_bf)
        onesJ = sing.tile([P, P], BF16)
        nc.vector.memset(onesJ, 1.0 / D)
        onesJf = sing.tile([P, P], F32)
        nc.vector.memset(onesJf, 1.0 / D)
        onesJr = onesJf.bitcast(F32R)
        ones1_f = sing.tile([1, P], F32)
        nc.vector.memset(ones1_f, 1.0)
        ones1_r = sing.tile([1, P], F32R)
        nc.vector.tensor_copy(ones1_r, ones1_f)
        eps_t = sing.tile([P, 1], F32)
        nc.vector.memset(eps_t, EPS)
        zero16 = sing.tile([P, 16], BF16)
        nc.vector.memset(zero16, 0.0)

        def load_small(name, shape, dt=F32):
            t = sing.tile(list(shape), dt, name=f"sb_{name}")
            nc.sync.dma_start(t[:], pr[name][:])
            return t

        sb_bff11 = load_small("b_ff1_1", [P, KF])
        sb_bff12 = load_small("b_ff1_2", [P, KD])
        sb_bff21 = load_small("b_ff2_1", [P, KF])
        sb_bff22 = load_small("b_ff2_2", [P, KD])
        sb_qkvb = load_small("qkv_b", [P, 8])
        sb_qkvbv = load_small("qkv_bv", [1, INNER], F32R)
        sb_bpw1 = load_small("b_pw1", [P, 2 * KC])
        sb_dww = load_small("dw_w", [P, KC, KW])
        sb_dwb = load_small("dw_b", [P, KC])
        sb_bpw2 = load_small("b_pw2", [P, KD])
        sb_postg = load_small("post_g", [P, KD])
        sb_postb = load_small("post_b", [P, KD])
        sb_rmat = load_small("r_mat", [P, 1024], BF16)

        def load_w(ap, ktiles, fdim, tag, name):
            """Load [ktiles*P, fdim] weights as two half-k tiles (smaller pool
            slots so double-buffered prefetch fits SBUF). Returns an indexable
            shim: w[:, k, a:b]."""
            kh = ktiles // 2
            src = ap.rearrange("(k p) f -> p k f", p=P)
            halves = []
            for hlf in range(2):
                t = st([P, kh, fdim], BF16, tag, 3, f"{name}_{hlf}")
                for k in range(kh):
                    nc.scalar.dma_start(t[:, k, :], src[:, hlf * kh + k, :])
                halves.append(t)

            class _W:
                def __getitem__(self, idx):
                    _, k, fs = idx
                    return halves[k // kh][:, k % kh, fs]
            return _W()

        # ---- input ----
        x = {}
        for b in range(BPC):
            tiles = []
            for k in range(KD):
                t = st([P, NT], F32R, "xcur", 10, f"x0_{b}_{k}")
                nc.sync.dma_start(t[:], pr["xT"][b, k * P:(k + 1) * P, :])
                tiles.append(t)
            x[b] = tiles

        # ---- layernorm: broadcast-first stats (both batches together so the
        # scalar engine does not thrash activation tables) ----
        def ln_pair(pfx):
            sqs, mbs, rinvs, vars_ = {}, {}, {}, {}
            for b in range(BPC):
                sqs[b] = []
                for k in range(KD):
                    s = st([P, NT], BF16, "sq", 6, f"sq{pfx}{b}_{k}")
                    nc.scalar.activation(s, x[b][k].bitcast(F32), AF.Square)
                    sqs[b].append(s)
            for b in range(BPC):
                mean_ps = mm_ps(f"mean{pfx}{b}")
                for k in range(KD):
                    nc.tensor.matmul(mean_ps, onesJr, x[b][k],
                                     start=(k == 0), stop=(k == KD - 1))
                ex2_ps = mm_ps(f"ex2{pfx}{b}")
                for k in range(KD):
                    nc.tensor.matmul(ex2_ps, onesJ, sqs[b][k],
                                     start=(k == 0), stop=(k == KD - 1))
                mb = st([P, NT], BF16, "mb", 2, f"mb{pfx}{b}")
                nc.vector.tensor_copy(mb, mean_ps)
                m2 = st([P, NT], BF16, "m2", 2, f"m2{pfx}{b}")
                nc.vector.tensor_mul(m2, mb, mb)
                var = st([P, NT], F32, "var", 2, f"var{pfx}{b}")
                nc.vector.tensor_tensor(var, ex2_ps, m2, OP.subtract)
                mbs[b], vars_[b] = mb, var
            for b in range(BPC):
                nc.scalar.activation(vars_[b], vars_[b], AF.Sqrt,
                                     bias=eps_t[:, 0:1])
            for b in range(BPC):
                rinv = st([P, NT], F32, "rinv", 2, f"rinv{pfx}{b}")
                nc.vector.reciprocal_approx_fast(rinv, vars_[b])
                rinvs[b] = rinv
            return mbs, rinvs

        def ln_apply(b, mb, rinv, pfx):
            hs = []
            for k in range(KD):
                t = st([P, NT], BF16, "t2k", 3, f"t{pfx}{b}_{k}")
                nc.vector.tensor_tensor(t, x[b][k].bitcast(F32), mb, OP.subtract)
                h = st([P, NT], BF16, "h", 8, f"h{pfx}{b}_{k}")
                nc.vector.tensor_mul(h, t, rinv)
                hs.append(h)
            return hs

        # ---- feed-forward ----
        def ff_stage(w1name, w2name, b1t, b2t, pfx):
            w1 = load_w(pr[w1name], KD, FF, "wbig", f"w1{pfx}")
            w2 = load_w(pr[w2name], KF, D, "wmid", f"w2{pfx}")
            mbs, rinvs = ln_pair(pfx)
            hh = {}
            for b in range(BPC):
                hh[b] = ln_apply(b, mbs[b], rinvs[b], pfx)
            for b in range(BPC):
                hs = hh[b]
                y1 = st([P, KF, NT], BF16, "y1s", 1, f"y1s{pfx}{b}")
                for f in range(KF):
                    ps = mm_ps(f"ps1{pfx}{b}_{f}")
                    for k in range(KD):
                        nc.tensor.matmul(ps, w1[:, k, f * P:(f + 1) * P], hs[k],
                                         start=(k == 0), stop=(k == KD - 1))
                    nc.scalar.activation(y1[:, f, :], ps, AF.Silu,
                                         bias=b1t[:, f:f + 1])
                newx = []
                for f in range(KD):
                    ps = mm_ps(f"ps2{pfx}{b}_{f}")
                    for k in range(KF):
                        nc.tensor.matmul(ps, w2[:, k, f * P:(f + 1) * P],
                                         y1[:, k, :],
                                         start=(k == 0), stop=(k == KF - 1))
                    nx = st([P, NT], F32R, "xcur", 10, f"x{pfx}{b}_{f}")
                    nc.vector.scalar_tensor_tensor(
                        nx, ps, b2t[:, f:f + 1], x[b][f].bitcast(F32),
                        OP.add, OP.add)
                    newx.append(nx)
                x[b] = newx

        # ---- attention ----
        def attn_stage():
            wq = load_w(pr["qkv_w"], KD, 3 * INNER, "wbig", "wqkv")
            wo = load_w(pr["out_w"], KD, D, "wmid", "wout")
            mbs, rinvs = ln_pair("at")
            hh = {}
            for b in range(BPC):
                hh[b] = ln_apply(b, mbs[b], rinvs[b], "at")
            for b in range(BPC):
                hs = hh[b]
                qk = st([P, 8, NT], BF16, "qk", 1, f"qk{b}")
                for f in range(8):
                    ps = mm_ps(f"qkps{b}_{f}")
                    for k in range(KD):
                        nc.tensor.matmul(ps, wq[:, k, f * P:(f + 1) * P], hs[k],
                                         start=(k == 0), stop=(k == KD - 1))
                    nc.vector.tensor_scalar(qk[:, f, :], ps,
                                            sb_qkvb[:, f:f + 1], None, OP.add)
                vt = st([P, KD, INNER], BF16, "vt", 2, f"vt{b}")
                for n in range(KD):
                    ps = mm_ps(f"vps{b}_{n}")
                    for k in range(KD):
                        nc.tensor.matmul(ps, hs[k][:, n * P:(n + 1) * P],
                                         wq[:, k, 2 * INNER:3 * INNER],
                                         start=(k == 0), stop=False)
                    nc.tensor.matmul(ps, ones1_r[0:1, :], sb_qkvbv,
                                     start=False, stop=True)
                    nc.vector.tensor_copy(vt[:, n, :], ps)
                ao = st([P, KD, NT], BF16, "ao", 2, f"ao{b}")
                for hp in range(4):
                    qt = qk[:, hp, :]
                    kt = qk[:, 4 + hp, :]
                    # --- rel-pos S over 640-wide windows, staged via DRAM ---
                    Sd = {}
                    for eo in range(2):
                        Sd[eo] = dram.tile([NT, SW], BF16, tag="Sd",
                                           name=f"Sd{b}_{hp}_{eo}")
                    for mi in range(KD):
                        w0 = 384 - 128 * mi
                        for eo in range(2):
                            po = eo * DH
                            sm = mm_ps(f"sm{b}_{hp}_{mi}_{eo}")
                            nc.tensor.matmul(
                                sm, qt[po:po + DH, mi * P:(mi + 1) * P],
                                sb_rmat[po:po + DH, w0:w0 + 512],
                                start=True, stop=True)
                            se = pt([P, SW - 512], F32, "mm", 4,
                                    f"se{b}_{hp}_{mi}_{eo}")
                            nc.tensor.matmul(
                                se, qt[po:po + DH, mi * P:(mi + 1) * P],
                                sb_rmat[po:po + DH, w0 + 512:w0 + SW],
                                start=True, stop=True)
                            sbf = st([P, SW], BF16, "sbf", 2,
                                     f"sbf{b}_{hp}_{mi}_{eo}")
                            nc.scalar.activation(sbf[:, :512], sm, AF.Copy)
                            nc.scalar.activation(sbf[:, 512:], se, AF.Copy)
                            nc.sync.dma_start(
                                Sd[eo][mi * P:(mi + 1) * P, :], sbf[:])
                    # --- dots + softmax (heads run in 64-row PE tiles) ---
                    attn = {}
                    sums = {}
                    for eo in range(2):
                        attn[eo] = st([P, KD, NT], BF16, "attn", 2,
                                      f"at{b}_{hp}_{eo}")
                        sums[eo] = st([P, KD], F32, "sums", 6,
                                      f"sums{b}_{hp}_{eo}")
                    for mi in range(KD):
                        dps = {}
                        for eo in range(2):
                            po = eo * DH
                            dp = mm_ps(f"dots{b}_{hp}_{mi}_{eo}")
                            nc.tensor.matmul(
                                dp, qt[po:po + DH, mi * P:(mi + 1) * P],
                                kt[po:po + DH, :], start=True, stop=True)
                            dps[eo] = dp
                        for eo in range(2):
                            pos = st([P, NT], BF16, "pos", 3,
                                     f"pos{b}_{hp}_{mi}_{eo}")
                            skew = bass.AP(
                                tensor=Sd[eo].tensor,
                                offset=Sd[eo].offset + mi * P * SW + 127,
                                ap=[[SW - 1, P], [1, NT]])
                            nc.sync.dma_start(pos[:], skew)
                            sc = st([P, NT], BF16, "sc", 2,
                                    f"sc{b}_{hp}_{mi}_{eo}")
                            nc.vector.tensor_tensor(sc, dps[eo], pos, OP.add)
                            nc.scalar.activation(
                                attn[eo][:, mi, :], sc, AF.Exp,
                                accum_out=sums[eo][:, mi:mi + 1])
                    opsb = pt([P, NT], F32, "av", 2, f"ops{b}_{hp}")
                    for eo in range(2):
                        po = eo * DH
                        h = 2 * hp + eo
                        rec = st([P, KD], F32, "sums", 6, f"rec{b}_{hp}_{eo}")
                        nc.vector.reciprocal_approx_fast(rec, sums[eo])
                        for mi in range(KD):
                            nc.vector.tensor_scalar_mul(
                                attn[eo][:, mi, :], attn[eo][:, mi, :],
                                rec[:, mi:mi + 1])
                        attT = st([P, KD, NT], BF16, "attT", 2,
                                  f"attT{b}_{hp}_{eo}")
                        for ki in range(KD):
                            tps = pt([P, NT], BF16, "tr", 2,
                                     f"tr{b}_{hp}_{eo}_{ki}")
                            for mi in range(KD):
                                nc.tensor.transpose(
                                    tps[:, mi * P:(mi + 1) * P],
                                    attn[eo][:, mi, ki * P:(ki + 1) * P],
                                    ident_bf)
                            nc.vector.tensor_copy(attT[:, ki, :], tps)
                            nc.tensor.matmul(
                                opsb[po:po + DH, :],
                                vt[:, ki, h * DH:(h + 1) * DH],
                                attT[:, ki, :],
                                start=(ki == 0), stop=(ki == KD - 1))
                    nc.vector.tensor_copy(ao[:, hp, :], opsb)
                newx = []
                for f in range(KD):
                    ps = mm_ps(f"oproj{b}_{f}")
                    for k in range(KD):
                        nc.tensor.matmul(ps, wo[:, k, f * P:(f + 1) * P],
                                         ao[:, k, :],
                                         start=(k == 0), stop=(k == KD - 1))
                    nx = st([P, NT], F32R, "xcur", 10, f"xat{b}_{f}")
                    nc.vector.tensor_tensor(nx, ps, x[b][f].bitcast(F32), OP.add)
                    newx.append(nx)
                x[b] = newx

        # ---- conv module ----
        def conv_stage():
            w1 = load_w(pr["pw1_wT"], KD, 2 * CI, "wbig", "wpw1")
            w2 = load_w(pr["pw2_wT"], KC, D, "wmid", "wpw2")
            mbs, rinvs = ln_pair("cv")
            hh = {}
            for b in range(BPC):
                hh[b] = ln_apply(b, mbs[b], rinvs[b], "cv")
            hc = {b: st([P, KC, NT], BF16, "hc", 2, f"hc{b}")
                  for b in range(BPC)}
            for c in range(KC):
                hg = {}
                for b in range(BPC):
                    hs = hh[b]
                    hgb = st([P, HGW], BF16, "hglu", 6, f"hglu{b}_{c}")
                    nc.vector.tensor_copy(hgb[:, 0:15], zero16[:, 0:15])
                    nc.vector.tensor_copy(hgb[:, NT + 15:], zero16[:, 0:HGW - NT - 15])
                    pso = mm_ps(f"glo{b}_{c}")
                    for k in range(KD):
                        nc.tensor.matmul(pso, w1[:, k, c * P:(c + 1) * P], hs[k],
                                         start=(k == 0), stop=(k == KD - 1))
                    psg = mm_ps(f"glg{b}_{c}")
                    for k in range(KD):
                        nc.tensor.matmul(psg,
                                         w1[:, k, CI + c * P:CI + (c + 1) * P],
                                         hs[k],
                                         start=(k == 0), stop=(k == KD - 1))
                    sg = st([P, NT], BF16, "sg", 3, f"sig{b}_{c}")
                    nc.scalar.activation(sg, psg, AF.Sigmoid,
                                         bias=sb_bpw1[:, KC + c:KC + c + 1])
                    nc.vector.scalar_tensor_tensor(
                        hgb[:, 15:NT + 15], pso, sb_bpw1[:, c:c + 1], sg,
                        OP.add, OP.mult)
                    hg[b] = hgb
                diags = {}
                for i, k in enumerate(PE_TAPS):
                    d = st([P, P], BF16, "diag", 33, f"dg{c}_{k}")
                    if i % 2 == 0:
                        nc.scalar.activation(d, ident_bf, AF.Copy,
                                             scale=sb_dww[:, c, k:k + 1])
                    else:
                        nc.vector.tensor_scalar_mul(d, ident_bf,
                                                    sb_dww[:, c, k:k + 1])
                    diags[k] = d
                for b in range(BPC):
                    hgb = hg[b]
                    ps = mm_ps(f"cv{b}_{c}")
                    for i, k in enumerate(PE_TAPS):
                        nc.tensor.matmul(ps, diags[k], hgb[:, k:k + NT],
                                         start=(i == 0),
                                         stop=(i == len(PE_TAPS) - 1))
                    nc.scalar.activation(hc[b][:, c, :], ps, AF.Silu,
                                         bias=sb_dwb[:, c:c + 1])
            for b in range(BPC):
                newx = []
                for f in range(KD):
                    ps = mm_ps(f"pw2{b}_{f}")
                    for k in range(KC):
                        nc.tensor.matmul(ps, w2[:, k, f * P:(f + 1) * P],
                                         hc[b][:, k, :],
                                         start=(k == 0), stop=(k == KC - 1))
                    nx = st([P, NT], F32R, "xcur", 10, f"xcv{b}_{f}")
                    nc.vector.scalar_tensor_tensor(
                        nx, ps, sb_bpw2[:, f:f + 1], x[b][f].bitcast(F32),
                        OP.add, OP.add)
                    newx.append(nx)
                x[b] = newx

        # ---- post layernorm ----
        def post_stage():
            mbs, rinvs = ln_pair("po")
            for b in range(BPC):
                mb, rinv = mbs[b], rinvs[b]
                for f in range(KD):
                    t = st([P, NT], BF16, "t2k", 3, f"pt{b}_{f}")
                    nc.vector.tensor_tensor(t, x[b][f].bitcast(F32), mb,
                                            OP.subtract)
                    t2 = st([P, NT], BF16, "sg", 3, f"pt2{b}_{f}")
                    nc.vector.tensor_mul(t2, t, rinv)
                    yt = st([P, NT], F32, "yout", 2, f"y{b}_{f}")
                    nc.vector.tensor_scalar(yt, t2, sb_postg[:, f:f + 1],
                                            sb_postb[:, f:f + 1],
                                            OP.mult, OP.add)
                    nc.sync.dma_start(pr["yO"][b, f * P:(f + 1) * P, :], yt[:])

        def tap(i):
            if not DEBUG_TAPS:
                return
            for b in range(BPC):
                for f in range(KD):
                    nc.sync.dma_start(pr[f"dbg{i}"][b, f * P:(f + 1) * P, :],
                                      x[b][f].bitcast(F32)[:])

        ff_stage("w_ff1_1", "w_ff1_2", sb_bff11, sb_bff12, "f1")
        tap(1)
        attn_stage()
        tap(2)
        conv_stage()
        tap(3)
        ff_stage("w_ff2_1", "w_ff2_2", sb_bff21, sb_bff22, "f2")
        tap(4)
        post_stage()


# --------------------------------------------------------------------------
# host entry point
# --------------------------------------------------------------------------

_NC = None


def _get_nc():
    global _NC
    if _NC is None:
        _NC = _build_nc()
    return _NC


def _shared_maps(p):
    return {
        'w_ff1_1': p['w_ff1_1'].astype(BFNP),
        'w_ff1_2': p['w_ff1_2'].astype(BFNP),
        'w_ff2_1': p['w_ff2_1'].astype(BFNP),
        'w_ff2_2': p['w_ff2_2'].astype(BFNP),
        'qkv_w': p['qkv_w'].astype(BFNP),
        'out_w': p['out_w'].astype(BFNP),
        'r_mat': p['r_mat'].astype(BFNP),
        'pw1_wT': p['pw1_wT'].astype(BFNP),
        'pw2_wT': p['pw2_wT'].astype(BFNP),
        'qkv_bv': p['qkv_bv'],
        'b_ff1_1': _col(p['b_ff1_1'], KF), 'b_ff1_2': _col(p['b_ff1_2'], KD),
        'b_ff2_1': _col(p['b_ff2_1'], KF), 'b_ff2_2': _col(p['b_ff2_2'], KD),
        'qkv_b': _col(p['qkv_b'][:2 * INNER], 8),
        'b_pw1': _col(p['b_pw1'], 2 * KC),
        'dw_w': np.ascontiguousarray(
            p['dw_w'].reshape(KC, P, KW).transpose(1, 0, 2)),
        'dw_b': _col(p['dw_b'], KC),
        'b_pw2': _col(p['b_pw2'], KD),
        'post_g': _col(p['post_g'], KD), 'post_b': _col(p['post_b'], KD),
    }


def kernel(**inputs):
    p = _host_prepare(inputs)
    x = np.asarray(inputs['x'], np.float32)
    shared = _shared_maps(p)
    in_maps = []
    for c in range(NCORES):
        m = dict(shared)
        xb = x[c * BPC:(c + 1) * BPC]
        m['xT'] = np.ascontiguousarray(xb.transpose(0, 2, 1))
        in_maps.append(m)

    nc = _get_nc()
    res = run_bass_kernel_spmd(nc, in_maps, core_ids=list(range(NCORES)))
    out = np.empty((B, NT, D), np.float32)
    for c in range(NCORES):
        yT = res.results[c]['y']
        out[c * BPC:(c + 1) * BPC] = yT.transpose(0, 2, 1)
    return out
